# revision 4
# baseline (speedup 1.0000x reference)
"""Trainium2 Bass kernel for DefaultKVCache attention (GQA decode-chunk).

Full-input contract: kernel(**inputs) takes the unsharded numpy inputs and
returns the full (B, NUM, H*HS) float32 output.

Problem shape (hardcoded):
  B=4, H=32, G=8 query groups (GQA 4 q-heads/group), HS=128,
  NUM=16 new tokens, cache length L=8192, input_pos (typically 4096).

Sharding: (batch, group-half) across 8 cores: core c -> b=c//2,
groups 4*(c%2)..4*(c%2)+4.  Fully local attention, no collectives.

Design (v2) — transposed-score orientation, host-side layout prep:
  - Host uploads K^T per group ([HS, T], new chunk concatenated) and V in
    SBUF-tiled layout ([t%128, t//128, HS], zero-padded), plus q^T.  All
    heavy operands are STATIONARY-side matmul inputs, so no on-device
    transposes of K or of the attention matrix are needed:
      S^T[t,qi]  = matmul(lhsT=K^T tile [h,t], rhs=q [h,qi])   (PSUM, f32)
      attn^T     = exp(scale*S^T)                              (Act -> f16)
      den[qi]    = matmul(lhsT=attn^T tile, rhs=ones [t,1])    (PSUM acc)
      out^T[h,qi]= matmul(lhsT=V tile [t,h], rhs=attn^T tile)  (PSUM acc)
    Final: transpose out^T via PE, multiply by 1/den per qi row, DMA out.
  - Only the last 16 cache rows need the causal mask (applied on the PSUM
    S^T tail tile with a DVE add before exp).
  - dtypes: q/attn/K fp16, V fp8-e3m4 with a 2x pre-scale folded into the
    softmax denominator (ones value = VSCALE).  Keeps rel-err ~1.2e-2 while
    halving V's DMA bytes.
"""
import sys
import numpy as np

for _p in ("/opt/trn_rl_repo", "/root/.axon_site/_ro/trn_rl_repo"):
    if _p not in sys.path:
        sys.path.insert(0, _p)

import ml_dtypes
from contextlib import ExitStack

import jax
from jax.sharding import Mesh, PartitionSpec
from jax.experimental.shard_map import shard_map

import concourse.bass as bass
from concourse import bacc, mybir, tile
import concourse.bass2jax as b2j

B, H, G, HS = 4, 32, 8, 128
NUM = 16
N_CORES = 8
NG = 4            # groups per core
QI = 64           # queries per group (4 heads x 16 tokens)
F32 = mybir.dt.float32
F16 = mybir.dt.float16
F8 = mybir.dt.float8e3       # e3m4
NEG = -1e30
EXP = mybir.ActivationFunctionType.Exp

# dtype knobs: "f16" or "f8" (fp8-e3m4, cast with the given pre-scale).
K_CFG = ("f16", 1.0)
V_CFG = ("f8", 2.0)

_DT = {"f16": (F16, np.float16), "f8": (F8, ml_dtypes.float8_e3m4)}


def build_program(pos):
    assert pos % 128 == 0 and NUM == 16
    T = pos + NUM
    n_full = pos // 128            # full 128-row K/V tiles
    n_vt = n_full + 1              # V tiles incl zero-padded tail tile
    scale = float(HS) ** -0.5
    kdt, _ = _DT[K_CFG[0]]
    vdt, _ = _DT[V_CFG[0]]

    nc = bacc.Bacc("TRN2", target_bir_lowering=False, debug=False,
                   enable_asserts=False, num_devices=N_CORES)
    kT = nc.dram_tensor("kT", [NG, HS, T], kdt, kind="ExternalInput").ap()
    vt = nc.dram_tensor("vt", [NG, 128, n_vt, HS], vdt,
                        kind="ExternalInput").ap()
    qT = nc.dram_tensor("qT", [HS, NG * QI], F16, kind="ExternalInput").ap()
    ident = nc.dram_tensor("ident", [128, 128], F16, kind="ExternalInput").ap()
    maskb = nc.dram_tensor("maskb", [NUM, QI], F32, kind="ExternalInput").ap()
    out = nc.dram_tensor("out", [NG, QI, HS], F32, kind="ExternalOutput").ap()

    with tile.TileContext(nc) as tc, ExitStack() as ctx:
        cpool = ctx.enter_context(tc.tile_pool(name="consts", bufs=1))
        apool = ctx.enter_context(tc.tile_pool(name="attn", bufs=4))
        npool = ctx.enter_context(tc.tile_pool(name="norm", bufs=2))
        ps_s = ctx.enter_context(tc.tile_pool(name="ps_s", bufs=3, space="PSUM"))
        ps_st = ctx.enter_context(tc.tile_pool(name="ps_st", bufs=1, space="PSUM"))
        ps_ot = ctx.enter_context(tc.tile_pool(name="ps_ot", bufs=1, space="PSUM"))
        ps_pv = ctx.enter_context(tc.tile_pool(name="ps_pv", bufs=2, space="PSUM"))
        ps_dn = ctx.enter_context(tc.tile_pool(name="ps_dn", bufs=1, space="PSUM"))

        # small constants (SP/HWDGE queue)
        q_sb = cpool.tile([HS, NG * QI], F16, tag="q")
        nc.sync.dma_start(q_sb[:], qT[:])
        mb_sb = cpool.tile([NUM, QI], F32, tag="mb")
        nc.sync.dma_start(mb_sb[:], maskb[:])
        id_sb = cpool.tile([128, 128], F16, tag="id")
        nc.sync.dma_start(id_sb[:], ident[:])
        ones = cpool.tile([128, 1], F16, tag="ones")
        nc.vector.memset(ones[:, :], float(V_CFG[1]))
        out_sb = cpool.tile([QI, NG, HS], F32, tag="out")

        # heavy loads: K^T halves and V halves per group, interleaved so
        # group g's data is complete before group g+1's
        kt_sb, v_sb = [], []
        for g in range(NG):
            kt_sb.append(cpool.tile([HS, T], kdt, tag="kt", name=f"kt{g}"))
            v_sb.append(cpool.tile([128, n_vt, HS], vdt, tag="v", name=f"v{g}"))
        for g in range(NG):
            h0 = (T // 2) // 16 * 16
            nc.sync.dma_start(kt_sb[g][:, :h0], kT[g, :, :h0])
            nc.sync.dma_start(kt_sb[g][:, h0:], kT[g, :, h0:])
            j0 = n_vt // 2
            nc.sync.dma_start(v_sb[g][:, :j0], vt[g, :, :j0])
            nc.sync.dma_start(v_sb[g][:, j0:], vt[g, :, j0:])

        # chunk list: (group, first_tile, n_full_tiles, is_tail)
        chunks = []
        for g in range(NG):
            for j0 in range(0, n_full, 8):
                chunks.append((g, j0, min(8, n_full - j0), False))
            chunks.append((g, n_full, 0, True))

        den_ps = [None] * NG
        pv_ps = [None] * NG

        def emit_qk(c):
            g, j0, nt, is_tail = chunks[c]
            if is_tail:
                spt = ps_st.tile([NUM, QI], F32, tag="st", name=f"st{c}")
                nc.tensor.matmul(spt[:, :], kt_sb[g][:, pos:pos + NUM],
                                 q_sb[:, g * QI:(g + 1) * QI],
                                 start=True, stop=True)
                nc.vector.tensor_add(spt[:, :], spt[:, :], mb_sb[:, :])
                return spt
            spt = ps_s.tile([128, nt * QI], F32, tag="s", name=f"s{c}")
            for j in range(nt):
                nc.tensor.matmul(spt[:, j * QI:(j + 1) * QI],
                                 kt_sb[g][:, (j0 + j) * 128:(j0 + j + 1) * 128],
                                 q_sb[:, g * QI:(g + 1) * QI],
                                 start=True, stop=True)
            return spt

        def emit_exp(c, spt):
            g, j0, nt, is_tail = chunks[c]
            if is_tail:
                at = apool.tile([NUM, QI], F16, tag="at", name=f"at{c}")
                nc.scalar.activation(at[:, :], spt[:, :], EXP, scale=scale)
            else:
                at = apool.tile([128, nt * QI], F16, tag="a", name=f"a{c}")
                nc.scalar.activation(at[:, :], spt[:, :], EXP, scale=scale)
            return at

        def emit_dv(c, at):
            g, j0, nt, is_tail = chunks[c]
            first = j0 == 0 and not is_tail
            if den_ps[g] is None:
                den_ps[g] = ps_dn.tile([QI, 1], F32, tag="dn", name=f"dn{g}")
                pv_ps[g] = ps_pv.tile([HS, QI], F32, tag="pv", name=f"pv{g}")
            if is_tail:
                nc.tensor.matmul(den_ps[g][:, :], at[:, :], ones[:NUM, :],
                                 start=False, stop=True, skip_group_check=True)
                nc.tensor.matmul(pv_ps[g][:, :], v_sb[g][:NUM, n_full, :],
                                 at[:, :], start=False, stop=True,
                                 skip_group_check=True)
                return
            for j in range(nt):
                st = first and j == 0
                nc.tensor.matmul(den_ps[g][:, :],
                                 at[:, j * QI:(j + 1) * QI], ones[:, :],
                                 start=st, stop=False,
                                 skip_group_check=not st)
                nc.tensor.matmul(pv_ps[g][:, :], v_sb[g][:, j0 + j, :],
                                 at[:, j * QI:(j + 1) * QI],
                                 start=st, stop=False,
                                 skip_group_check=True)

        def emit_norm(g):
            osb = npool.tile([HS, QI], F16, tag="os")
            nc.vector.tensor_copy(osb[:, :], pv_ps[g][:, :])
            ot = ps_ot.tile([QI, HS], F16, tag="ot", name=f"ot{g}")
            nc.tensor.transpose(ot[:, :], osb[:, :], id_sb[:, :])
            rec = npool.tile([QI, 1], F32, tag="rc")
            nc.vector.reciprocal(rec[:, :], den_ps[g][:, :])
            nc.vector.tensor_scalar_mul(out_sb[:, g, :], ot[:, :], rec[:, :])

        # software pipeline: keep 2 chunks of QK ahead of den/PV
        DEPTH = 2
        live = {}
        for c in range(len(chunks)):
            live[c] = emit_qk(c)
            live[c] = (live[c], emit_exp(c, live[c]))
            if c >= DEPTH:
                cc = c - DEPTH
                emit_dv(cc, live.pop(cc)[1])
                if chunks[cc][3]:
                    emit_norm(chunks[cc][0])
        for cc in sorted(live):
            emit_dv(cc, live.pop(cc)[1])
            if chunks[cc][3]:
                emit_norm(chunks[cc][0])

        nc.sync.dma_start(out.rearrange("g q h -> q g h"), out_sb[:, :, :])

    nc.compile()
    return nc


class _Runner:
    def __init__(self, nc):
        b2j.install_neuronx_cc_hook()
        self.nc = nc
        in_names, out_names, out_avals, zero_outs = [], [], [], []
        for alloc in nc.m.functions[0].allocations:
            if not isinstance(alloc, mybir.MemoryLocationSet):
                continue
            name = alloc.memorylocations[0].name
            if alloc.kind == "ExternalInput":
                in_names.append(name)
            elif alloc.kind == "ExternalOutput":
                out_names.append(name)
                shape = tuple(alloc.tensor_shape)
                dtype = mybir.dt.np(alloc.dtype)
                out_avals.append(jax.core.ShapedArray(shape, dtype))
                zero_outs.append(np.zeros(shape, dtype))
        part = nc.partition_id_tensor.name if nc.partition_id_tensor else None
        if part is not None:
            in_names = [n for n in in_names if n != part]
        self.in_names, self.out_names = in_names, out_names
        self.out_avals, self.zero_outs = out_avals, zero_outs
        all_names = in_names + out_names + ([part] if part else [])
        n_params = len(in_names)

        def _body(*args):
            operands = list(args)
            if part is not None:
                operands.append(b2j.partition_id_tensor())
            return tuple(b2j._bass_exec_p.bind(
                *operands, out_avals=tuple(out_avals), in_names=tuple(all_names),
                out_names=tuple(out_names), lowering_input_output_aliases=(),
                sim_require_finite=True, sim_require_nnan=True, nc=nc))

        devices = jax.devices()[:N_CORES]
        self.mesh = Mesh(np.asarray(devices), ("core",))
        in_specs = (PartitionSpec("core"),) * (n_params + len(out_names))
        out_specs = (PartitionSpec("core"),) * len(out_names)
        self.fn = jax.jit(shard_map(_body, mesh=self.mesh, in_specs=in_specs,
                                    out_specs=out_specs, check_rep=False),
                          keep_unused=True)

    def run(self, in_maps):
        sharding = jax.sharding.NamedSharding(self.mesh, PartitionSpec("core"))
        args = []
        for name in self.in_names:
            arr = np.concatenate([np.asarray(m[name]) for m in in_maps], axis=0)
            args.append(jax.device_put(arr, sharding))
        for z in self.zero_outs:
            args.append(jax.device_put(
                np.zeros((N_CORES * z.shape[0], *z.shape[1:]), z.dtype), sharding))
        outs = self.fn(*args)
        jax.block_until_ready(outs)
        return [{name: np.asarray(outs[i]).reshape(
            N_CORES, *self.out_avals[i].shape)[c]
            for i, name in enumerate(self.out_names)}
            for c in range(N_CORES)]


_cache = {}


def _get_runner(pos):
    if pos not in _cache:
        _cache[pos] = _Runner(build_program(pos))
    return _cache[pos]


def _make_maskb():
    # S^T tail tile [t_local, qi]: new token n=(qi%16) sees row p iff p<=n
    m = np.zeros((NUM, QI), np.float32)
    p = np.arange(NUM)[:, None]
    n = (np.arange(QI) % NUM)[None, :]
    m[p > n] = NEG
    return m


def _cast(x, cfg):
    dt, npdt = _DT[cfg[0]]
    s = cfg[1]
    if s != 1.0:
        x = np.asarray(x, np.float32) * s
    return np.asarray(x, np.float32).astype(npdt)


def kernel(query, key, value, k_cache, v_cache, input_pos):
    query = np.asarray(query, np.float32)
    key = np.asarray(key, np.float32)
    value = np.asarray(value, np.float32)
    k_cache = np.asarray(k_cache, np.float32)
    v_cache = np.asarray(v_cache, np.float32)
    pos = int(input_pos)
    T = pos + NUM
    n_vt = pos // 128 + 1

    runner = _get_runner(pos)
    ident = np.eye(128, dtype=np.float16)
    maskb = _make_maskb()

    in_maps = []
    for c in range(N_CORES):
        b = c // 2
        g0 = 4 * (c % 2)
        # q^T: [h, g, hd, tok] -> [HS, NG*QI]
        qs = query[b, g0 * 4:(g0 + NG) * 4]          # [16 heads, NUM, HS]
        qTh = np.ascontiguousarray(
            qs.reshape(NG * QI, HS).T).astype(np.float16)
        # K^T per group: [HS, T] with the new chunk appended
        kf = np.concatenate([k_cache[b, g0:g0 + NG, :pos], key[b, g0:g0 + NG]],
                            axis=1)                  # [NG, T, HS]
        kTh = _cast(np.ascontiguousarray(kf.transpose(0, 2, 1)), K_CFG)
        # V tiled: [NG, t%128, t//128, HS], zero-padded to n_vt*128 rows
        vf = np.concatenate([v_cache[b, g0:g0 + NG, :pos], value[b, g0:g0 + NG]],
                            axis=1)                  # [NG, T, HS]
        vp = np.zeros((NG, n_vt * 128, HS), np.float32)
        vp[:, :T] = vf
        vth = _cast(np.ascontiguousarray(
            vp.reshape(NG, n_vt, 128, HS).transpose(0, 2, 1, 3)), V_CFG)
        in_maps.append({"kT": kTh, "vt": vth, "qT": qTh,
                        "ident": ident, "maskb": maskb})

    results = runner.run(in_maps)

    full = np.empty((B, H, NUM, HS), np.float32)
    for c in range(N_CORES):
        b = c // 2
        g0 = 4 * (c % 2)
        full[b, g0 * 4:(g0 + NG) * 4] = results[c]["out"].reshape(16, NUM, HS)
    return np.ascontiguousarray(
        full.transpose(0, 2, 1, 3).reshape(B, NUM, H * HS))


# revision 13
# speedup vs baseline: 1.2649x; 1.2649x over previous
"""Trainium2 Bass kernel for DefaultKVCache attention (GQA decode-chunk).

Full-input contract: kernel(**inputs) takes the unsharded numpy inputs and
returns the full (B, NUM, H*HS) float32 output.

Problem shape (hardcoded):
  B=4, H=32, G=8 query groups (GQA 4 q-heads/group), HS=128,
  NUM=16 new tokens, cache length L=8192, input_pos (typically 4096).

Sharding: (batch, group-half) across 8 cores: core c -> b=c//2,
groups 4*(c%2)..4*(c%2)+4.  Fully local attention, no collectives.

Design (v2) — transposed-score orientation, host-side layout prep:
  - Host uploads K^T per group ([HS, T], new chunk concatenated) and V in
    SBUF-tiled layout ([t%128, t//128, HS], zero-padded), plus q^T.  All
    heavy operands are STATIONARY-side matmul inputs, so no on-device
    transposes of K or of the attention matrix are needed:
      S^T[t,qi]  = matmul(lhsT=K^T tile [h,t], rhs=q [h,qi])   (PSUM, f32)
      attn^T     = exp(scale*S^T)                              (Act -> f16)
      den[qi]    = matmul(lhsT=attn^T tile, rhs=ones [t,1])    (PSUM acc)
      out^T[h,qi]= matmul(lhsT=V tile [t,h], rhs=attn^T tile)  (PSUM acc)
    Final: transpose out^T via PE, multiply by 1/den per qi row, DMA out.
  - Only the last 16 cache rows need the causal mask (applied on the PSUM
    S^T tail tile with a DVE add before exp).
  - dtypes: q/attn/K fp16, V fp8-e3m4 with a 2x pre-scale folded into the
    softmax denominator (ones value = VSCALE).  Keeps rel-err ~1.2e-2 while
    halving V's DMA bytes.
"""
import sys
import numpy as np

for _p in ("/opt/trn_rl_repo", "/root/.axon_site/_ro/trn_rl_repo"):
    if _p not in sys.path:
        sys.path.insert(0, _p)

import ml_dtypes
from contextlib import ExitStack

import jax
from jax.sharding import Mesh, PartitionSpec
from jax.experimental.shard_map import shard_map

import concourse.bass as bass
from concourse import bacc, mybir, tile
import concourse.bass2jax as b2j

B, H, G, HS = 4, 32, 8, 128
NUM = 16
N_CORES = 8
NG = 4            # groups per core
QI = 64           # queries per group (4 heads x 16 tokens)
F32 = mybir.dt.float32
F16 = mybir.dt.float16
F8 = mybir.dt.float8e3       # e3m4
NEG = -1e30
EXP = mybir.ActivationFunctionType.Exp

# dtype knobs: "f16" or "f8" (fp8-e3m4, cast with the given pre-scale).
K_CFG = ("f16", 1.0)
V_CFG = ("f8", 2.0)

_DT = {"f16": (F16, np.float16), "f8": (F8, ml_dtypes.float8_e3m4)}


def build_program(pos):
    assert pos % 128 == 0 and NUM == 16
    T = pos + NUM
    n_full = pos // 128            # full 128-row K/V tiles
    n_vt = n_full + 1              # V tiles incl zero-padded tail tile
    scale = float(HS) ** -0.5
    kdt, _ = _DT[K_CFG[0]]
    vdt, _ = _DT[V_CFG[0]]

    nc = bacc.Bacc("TRN2", target_bir_lowering=False, debug=False,
                   enable_asserts=False, num_devices=N_CORES)
    kT = nc.dram_tensor("kT", [NG, HS, T], kdt, kind="ExternalInput").ap()
    vt = nc.dram_tensor("vt", [NG, 128, n_vt, HS], vdt,
                        kind="ExternalInput").ap()
    qT = nc.dram_tensor("qT", [HS, NG * QI], F16, kind="ExternalInput").ap()
    ident = nc.dram_tensor("ident", [128, 128], F16, kind="ExternalInput").ap()
    maskb = nc.dram_tensor("maskb", [NUM, QI], F32, kind="ExternalInput").ap()
    out = nc.dram_tensor("out", [NG, QI, HS], F32, kind="ExternalOutput").ap()

    with tile.TileContext(nc) as tc, ExitStack() as ctx:
        cpool = ctx.enter_context(tc.tile_pool(name="consts", bufs=1))
        apool = ctx.enter_context(tc.tile_pool(name="attn", bufs=4))
        npool = ctx.enter_context(tc.tile_pool(name="norm", bufs=2))
        ps_s = ctx.enter_context(tc.tile_pool(name="ps_s", bufs=3, space="PSUM"))
        ps_ot = ctx.enter_context(tc.tile_pool(name="ps_ot", bufs=1, space="PSUM"))
        ps_st = ctx.enter_context(tc.tile_pool(name="ps_st", bufs=1, space="PSUM"))
        ps_pv = ctx.enter_context(tc.tile_pool(name="ps_pv", bufs=2, space="PSUM"))
        ps_dn = ctx.enter_context(tc.tile_pool(name="ps_dn", bufs=1, space="PSUM"))

        # q first (needed by the first QK), then group 0's K/V, then the
        # remaining constants, then groups 1-3.  Every tile gets its own tag
        # so nothing shares a pool slot (a shared slot serializes the DMA
        # stream behind the previous group's compute).
        q_sb = cpool.tile([HS, NG * QI], F16, tag="q")
        ones = cpool.tile([128, 1], F16, tag="ones")
        nc.vector.memset(ones[:, :], float(V_CFG[1]))
        out_sb = cpool.tile([QI, NG, HS], F32, tag="out")

        # single SBUF tiles holding all 4 groups, so the tiny per-group tail
        # slices coalesce into ONE DMA each (every DMA instruction costs
        # ~625ns of serialized HWDGE descriptor-gen)
        kt_all = cpool.tile([HS, NG, T], kdt, tag="kt")
        v_all = cpool.tile([128, NG, n_vt, HS], vdt, tag="v")
        kt_sb = [kt_all[:, g] for g in range(NG)]
        v_sb = [v_all[:, g] for g in range(NG)]
        mb_sb = cpool.tile([NUM, QI], F32, tag="mb")
        id_sb = cpool.tile([128, 128], F16, tag="id")

        # small loads go on the Pool/SWDGE queue: their descriptor-gen runs
        # on the otherwise-idle Pool engine instead of the shared HWDGE.
        # Their transfers slot into DMA-engine gaps.
        nc.gpsimd.dma_start(mb_sb[:], maskb[:])
        nc.gpsimd.dma_start(kt_all[:, :, pos:],
                            kT[:, :, pos:].rearrange("g h t -> h g t"))
        nc.gpsimd.dma_start(v_all[:NUM, :, n_full, :],
                            vt[:, :NUM, n_full, :].rearrange("g p h -> p g h"))
        nc.gpsimd.dma_start(id_sb[:], ident[:])

        # bulk K/V halves per group on the SP/HWDGE queue; the stream ends
        # with a small V slice so the post-DMA dependency tail is short
        # Stream order: K^T always ~2 groups ahead of V so every group's
        # QK+exp completes during earlier transfers; the stream ends with
        # small V slices for the last group so the post-DMA tail is just
        # PV(last tiles) + normalize + out-DMA.
        half = (pos // 2) // 16 * 16
        jh = n_full // 2
        gl = NG - 1

        def kt_halves(g):
            nc.sync.dma_start(kt_sb[g][:, :half], kT[g, :, :half])
            if g == 0:
                nc.sync.dma_start(q_sb[:], qT[:])
            nc.sync.dma_start(kt_sb[g][:, half:pos], kT[g, :, half:pos])

        def v_halves(g):
            nc.sync.dma_start(v_sb[g][:, :jh], vt[g, :, :jh])
            nc.sync.dma_start(v_sb[g][:, jh:n_full], vt[g, :, jh:n_full])

        for g in range(NG):
            kt_halves(g)
        for g in range(NG - 1):
            v_halves(g)
        nc.sync.dma_start(v_sb[gl][:, :jh], vt[gl, :, :jh])
        nc.sync.dma_start(v_sb[gl][:, jh:n_full - 4], vt[gl, :, jh:n_full - 4])
        nc.sync.dma_start(v_sb[gl][:, n_full - 4:n_full],
                          vt[gl, :, n_full - 4:n_full])

        # chunk list: (group, first_tile, n_full_tiles, is_tail);
        # tail first per group (opens the accumulation chains)
        chunks = []
        for g in range(NG):
            chunks.append((g, n_full, 0, True))
            for j0 in range(0, n_full, 8):
                chunks.append((g, j0, min(8, n_full - j0), False))

        den_ps = [None] * NG
        pv_ps = [None] * NG

        def emit_qk(c):
            g, j0, nt, is_tail = chunks[c]
            if is_tail:
                spt = ps_st.tile([NUM, QI], F32, tag="st", name=f"st{c}")
                nc.tensor.matmul(spt[:, :], kt_sb[g][:, pos:pos + NUM],
                                 q_sb[:, g * QI:(g + 1) * QI],
                                 start=True, stop=True)
                nc.vector.tensor_add(spt[:, :], spt[:, :], mb_sb[:, :])
                return spt
            spt = ps_s.tile([128, nt * QI], F32, tag="s", name=f"s{c}")
            for j in range(nt):
                nc.tensor.matmul(spt[:, j * QI:(j + 1) * QI],
                                 kt_sb[g][:, (j0 + j) * 128:(j0 + j + 1) * 128],
                                 q_sb[:, g * QI:(g + 1) * QI],
                                 start=True, stop=True)
            return spt

        def emit_exp(c, spt):
            g, j0, nt, is_tail = chunks[c]
            if is_tail:
                at = apool.tile([NUM, QI], F16, tag=f"at{c}", name=f"at{c}")
            else:
                at = apool.tile([128, nt * QI], F16, tag=f"a{c}", name=f"a{c}")
            nc.scalar.activation(at[:, :], spt[:, :], EXP, scale=scale)
            return at

        def emit_dv(c, at):
            g, j0, nt, is_tail = chunks[c]
            if den_ps[g] is None:
                den_ps[g] = ps_dn.tile([QI, 1], F32, tag="dn", name=f"dn{g}")
                pv_ps[g] = ps_pv.tile([HS, QI], F32, tag="pv", name=f"pv{g}")
            if is_tail:
                nc.tensor.matmul(den_ps[g][:, :], at[:, :], ones[:NUM, :],
                                 start=True, stop=False)
                nc.tensor.matmul(pv_ps[g][:, :], v_sb[g][:NUM, n_full, :],
                                 at[:, :], start=True, stop=False,
                                 skip_group_check=True)
                return
            for j in range(nt):
                sp = j0 + j == n_full - 1
                nc.tensor.matmul(den_ps[g][:, :],
                                 at[:, j * QI:(j + 1) * QI], ones[:, :],
                                 start=False, stop=sp,
                                 skip_group_check=True)
                nc.tensor.matmul(pv_ps[g][:, :], v_sb[g][:, j0 + j, :],
                                 at[:, j * QI:(j + 1) * QI],
                                 start=False, stop=sp,
                                 skip_group_check=True)

        def emit_norm(g):
            rec = npool.tile([QI, 1], F32, tag="rc")
            nc.vector.reciprocal(rec[:, :], den_ps[g][:, :])
            osb = npool.tile([HS, QI], F16, tag="os")
            nc.vector.tensor_copy(osb[:, :], pv_ps[g][:, :])
            ot = ps_ot.tile([QI, HS], F16, tag="ot", name=f"ot{g}")
            nc.tensor.transpose(ot[:, :], osb[:, :], id_sb[:, :])
            nc.vector.tensor_scalar_mul(out_sb[:, g, :], ot[:, :], rec[:, :])
            nc.sync.dma_start(out[g], out_sb[:, g, :])

        # two-pass emission: every engine is in-order, so pass 1 (QK+exp,
        # paced by the K^T stream) must not interleave with pass 2 (den/PV,
        # paced by the V stream) or late-V stalls would block later QKs.
        ats = []
        for c in range(len(chunks)):
            ats.append(emit_exp(c, emit_qk(c)))
        for c in range(len(chunks)):
            emit_dv(c, ats[c])
            g, j0, nt, is_tail = chunks[c]
            if not is_tail and j0 + nt == n_full:
                emit_norm(g)

    nc.compile()
    return nc


class _Runner:
    def __init__(self, nc):
        b2j.install_neuronx_cc_hook()
        self.nc = nc
        in_names, out_names, out_avals, zero_outs = [], [], [], []
        for alloc in nc.m.functions[0].allocations:
            if not isinstance(alloc, mybir.MemoryLocationSet):
                continue
            name = alloc.memorylocations[0].name
            if alloc.kind == "ExternalInput":
                in_names.append(name)
            elif alloc.kind == "ExternalOutput":
                out_names.append(name)
                shape = tuple(alloc.tensor_shape)
                dtype = mybir.dt.np(alloc.dtype)
                out_avals.append(jax.core.ShapedArray(shape, dtype))
                zero_outs.append(np.zeros(shape, dtype))
        part = nc.partition_id_tensor.name if nc.partition_id_tensor else None
        if part is not None:
            in_names = [n for n in in_names if n != part]
        self.in_names, self.out_names = in_names, out_names
        self.out_avals, self.zero_outs = out_avals, zero_outs
        all_names = in_names + out_names + ([part] if part else [])
        n_params = len(in_names)

        def _body(*args):
            operands = list(args)
            if part is not None:
                operands.append(b2j.partition_id_tensor())
            return tuple(b2j._bass_exec_p.bind(
                *operands, out_avals=tuple(out_avals), in_names=tuple(all_names),
                out_names=tuple(out_names), lowering_input_output_aliases=(),
                sim_require_finite=True, sim_require_nnan=True, nc=nc))

        devices = jax.devices()[:N_CORES]
        self.mesh = Mesh(np.asarray(devices), ("core",))
        in_specs = (PartitionSpec("core"),) * (n_params + len(out_names))
        out_specs = (PartitionSpec("core"),) * len(out_names)
        self.fn = jax.jit(shard_map(_body, mesh=self.mesh, in_specs=in_specs,
                                    out_specs=out_specs, check_rep=False),
                          keep_unused=True)

    def run(self, in_maps):
        sharding = jax.sharding.NamedSharding(self.mesh, PartitionSpec("core"))
        args = []
        for name in self.in_names:
            arr = np.concatenate([np.asarray(m[name]) for m in in_maps], axis=0)
            args.append(jax.device_put(arr, sharding))
        for z in self.zero_outs:
            args.append(jax.device_put(
                np.zeros((N_CORES * z.shape[0], *z.shape[1:]), z.dtype), sharding))
        outs = self.fn(*args)
        jax.block_until_ready(outs)
        return [{name: np.asarray(outs[i]).reshape(
            N_CORES, *self.out_avals[i].shape)[c]
            for i, name in enumerate(self.out_names)}
            for c in range(N_CORES)]


_cache = {}


def _get_runner(pos):
    if pos not in _cache:
        _cache[pos] = _Runner(build_program(pos))
    return _cache[pos]


def _make_maskb():
    # S^T tail tile [t_local, qi]: new token n=(qi%16) sees row p iff p<=n
    m = np.zeros((NUM, QI), np.float32)
    p = np.arange(NUM)[:, None]
    n = (np.arange(QI) % NUM)[None, :]
    m[p > n] = NEG
    return m


def _cast(x, cfg):
    dt, npdt = _DT[cfg[0]]
    s = cfg[1]
    if s != 1.0:
        x = np.asarray(x, np.float32) * s
    return np.asarray(x, np.float32).astype(npdt)


def kernel(query, key, value, k_cache, v_cache, input_pos):
    query = np.asarray(query, np.float32)
    key = np.asarray(key, np.float32)
    value = np.asarray(value, np.float32)
    k_cache = np.asarray(k_cache, np.float32)
    v_cache = np.asarray(v_cache, np.float32)
    pos = int(input_pos)
    T = pos + NUM
    n_vt = pos // 128 + 1

    runner = _get_runner(pos)
    ident = np.eye(128, dtype=np.float16)
    maskb = _make_maskb()

    in_maps = []
    for c in range(N_CORES):
        b = c // 2
        g0 = 4 * (c % 2)
        # q^T: [h, g, hd, tok] -> [HS, NG*QI]
        qs = query[b, g0 * 4:(g0 + NG) * 4]          # [16 heads, NUM, HS]
        qTh = np.ascontiguousarray(
            qs.reshape(NG * QI, HS).T).astype(np.float16)
        # K^T per group: [HS, T] with the new chunk appended
        kf = np.concatenate([k_cache[b, g0:g0 + NG, :pos], key[b, g0:g0 + NG]],
                            axis=1)                  # [NG, T, HS]
        kTh = _cast(np.ascontiguousarray(kf.transpose(0, 2, 1)), K_CFG)
        # V tiled: [NG, t%128, t//128, HS], zero-padded to n_vt*128 rows
        vf = np.concatenate([v_cache[b, g0:g0 + NG, :pos], value[b, g0:g0 + NG]],
                            axis=1)                  # [NG, T, HS]
        vp = np.zeros((NG, n_vt * 128, HS), np.float32)
        vp[:, :T] = vf
        vth = _cast(np.ascontiguousarray(
            vp.reshape(NG, n_vt, 128, HS).transpose(0, 2, 1, 3)), V_CFG)
        in_maps.append({"kT": kTh, "vt": vth, "qT": qTh,
                        "ident": ident, "maskb": maskb})

    results = runner.run(in_maps)

    full = np.empty((B, H, NUM, HS), np.float32)
    for c in range(N_CORES):
        b = c // 2
        g0 = 4 * (c % 2)
        full[b, g0 * 4:(g0 + NG) * 4] = results[c]["out"].reshape(16, NUM, HS)
    return np.ascontiguousarray(
        full.transpose(0, 2, 1, 3).reshape(B, NUM, H * HS))


# revision 14
# speedup vs baseline: 1.5447x; 1.2211x over previous
"""Trainium2 Bass kernel for DefaultKVCache attention (GQA decode-chunk).

Full-input contract: kernel(**inputs) takes the unsharded numpy inputs and
returns the full (B, NUM, H*HS) float32 output.

Problem shape (hardcoded):
  B=4, H=32, G=8 query groups (GQA 4 q-heads/group), HS=128,
  NUM=16 new tokens, cache length L=8192, input_pos (typically 4096).

Sharding: (batch, group-half) across 8 cores: core c -> b=c//2,
groups 4*(c%2)..4*(c%2)+4.  Fully local attention, no collectives.

Design (v2) — transposed-score orientation, host-side layout prep:
  - Host uploads K^T per group ([HS, T], new chunk concatenated) and V in
    SBUF-tiled layout ([t%128, t//128, HS], zero-padded), plus q^T.  All
    heavy operands are STATIONARY-side matmul inputs, so no on-device
    transposes of K or of the attention matrix are needed:
      S^T[t,qi]  = matmul(lhsT=K^T tile [h,t], rhs=q [h,qi])   (PSUM, f32)
      attn^T     = exp(scale*S^T)                              (Act -> f16)
      den[qi]    = matmul(lhsT=attn^T tile, rhs=ones [t,1])    (PSUM acc)
      out^T[h,qi]= matmul(lhsT=V tile [t,h], rhs=attn^T tile)  (PSUM acc)
    Final: transpose out^T via PE, multiply by 1/den per qi row, DMA out.
  - Only the last 16 cache rows need the causal mask (applied on the PSUM
    S^T tail tile with a DVE add before exp).
  - dtypes: q/attn/K fp16, V fp8-e3m4 with a 2x pre-scale folded into the
    softmax denominator (ones value = VSCALE).  Keeps rel-err ~1.2e-2 while
    halving V's DMA bytes.
"""
import sys
import numpy as np

for _p in ("/opt/trn_rl_repo", "/root/.axon_site/_ro/trn_rl_repo"):
    if _p not in sys.path:
        sys.path.insert(0, _p)

import ml_dtypes
from contextlib import ExitStack

import jax
from jax.sharding import Mesh, PartitionSpec
from jax.experimental.shard_map import shard_map

import concourse.bass as bass
from concourse import bacc, mybir, tile
import concourse.bass2jax as b2j

B, H, G, HS = 4, 32, 8, 128
NUM = 16
N_CORES = 8
NG = 4            # groups per core
QI = 64           # queries per group (4 heads x 16 tokens)
F32 = mybir.dt.float32
F16 = mybir.dt.float16
F8 = mybir.dt.float8e3       # e3m4
NEG = -1e30
EXP = mybir.ActivationFunctionType.Exp

# dtype knobs: "f16" or "f8" (fp8-e3m4, cast with the given pre-scale).
K_CFG = ("f8", 1.5)
V_CFG = ("f8", 2.0)

_DT = {"f16": (F16, np.float16), "f8": (F8, ml_dtypes.float8_e3m4)}


def build_program(pos):
    assert pos % 128 == 0 and NUM == 16
    T = pos + NUM
    n_full = pos // 128            # full 128-row K/V tiles
    n_vt = n_full + 1              # V tiles incl zero-padded tail tile
    scale = float(HS) ** -0.5 / float(K_CFG[1])   # K pre-scale folds in here
    kdt, _ = _DT[K_CFG[0]]
    vdt, _ = _DT[V_CFG[0]]

    nc = bacc.Bacc("TRN2", target_bir_lowering=False, debug=False,
                   enable_asserts=False, num_devices=N_CORES)
    kT = nc.dram_tensor("kT", [NG, HS, T], kdt, kind="ExternalInput").ap()
    vt = nc.dram_tensor("vt", [NG, 128, n_vt, HS], vdt,
                        kind="ExternalInput").ap()
    qT = nc.dram_tensor("qT", [HS, NG * QI], F16, kind="ExternalInput").ap()
    ident = nc.dram_tensor("ident", [128, 128], F16, kind="ExternalInput").ap()
    maskb = nc.dram_tensor("maskb", [NUM, QI], F32, kind="ExternalInput").ap()
    out = nc.dram_tensor("out", [NG, QI, HS], F32, kind="ExternalOutput").ap()

    with tile.TileContext(nc) as tc, ExitStack() as ctx:
        cpool = ctx.enter_context(tc.tile_pool(name="consts", bufs=1))
        apool = ctx.enter_context(tc.tile_pool(name="attn", bufs=4))
        npool = ctx.enter_context(tc.tile_pool(name="norm", bufs=2))
        ps_s = ctx.enter_context(tc.tile_pool(name="ps_s", bufs=3, space="PSUM"))
        ps_ot = ctx.enter_context(tc.tile_pool(name="ps_ot", bufs=1, space="PSUM"))
        ps_st = ctx.enter_context(tc.tile_pool(name="ps_st", bufs=1, space="PSUM"))
        ps_pv = ctx.enter_context(tc.tile_pool(name="ps_pv", bufs=2, space="PSUM"))
        ps_dn = ctx.enter_context(tc.tile_pool(name="ps_dn", bufs=1, space="PSUM"))

        # q first (needed by the first QK), then group 0's K/V, then the
        # remaining constants, then groups 1-3.  Every tile gets its own tag
        # so nothing shares a pool slot (a shared slot serializes the DMA
        # stream behind the previous group's compute).
        q_sb = cpool.tile([HS, NG * QI], F16, tag="q")
        ones = cpool.tile([128, 1], F16, tag="ones")
        nc.vector.memset(ones[:, :], float(V_CFG[1]))
        out_sb = cpool.tile([QI, NG, HS], F32, tag="out")

        # single SBUF tiles holding all 4 groups, so the tiny per-group tail
        # slices coalesce into ONE DMA each (every DMA instruction costs
        # ~625ns of serialized HWDGE descriptor-gen)
        kt_all = cpool.tile([HS, NG, T], kdt, tag="kt")
        v_all = cpool.tile([128, NG, n_vt, HS], vdt, tag="v")
        kt_sb = [kt_all[:, g] for g in range(NG)]
        v_sb = [v_all[:, g] for g in range(NG)]
        mb_sb = cpool.tile([NUM, QI], F32, tag="mb")
        id_sb = cpool.tile([128, 128], F16, tag="id")

        # small loads go on the Pool/SWDGE queue: their descriptor-gen runs
        # on the otherwise-idle Pool engine instead of the shared HWDGE.
        # Their transfers slot into DMA-engine gaps.
        nc.gpsimd.dma_start(mb_sb[:], maskb[:])
        nc.gpsimd.dma_start(kt_all[:, :, pos:],
                            kT[:, :, pos:].rearrange("g h t -> h g t"))
        nc.gpsimd.dma_start(v_all[:NUM, :, n_full, :],
                            vt[:, :NUM, n_full, :].rearrange("g p h -> p g h"))
        nc.gpsimd.dma_start(id_sb[:], ident[:])

        # bulk K/V halves per group on the SP/HWDGE queue; the stream ends
        # with a small V slice so the post-DMA dependency tail is short
        # Stream order: K^T always ~2 groups ahead of V so every group's
        # QK+exp completes during earlier transfers; the stream ends with
        # small V slices for the last group so the post-DMA tail is just
        # PV(last tiles) + normalize + out-DMA.
        half = (pos // 2) // 16 * 16
        jh = n_full // 2
        gl = NG - 1

        def kt_halves(g):
            nc.sync.dma_start(kt_sb[g][:, :half], kT[g, :, :half])
            if g == 0:
                nc.sync.dma_start(q_sb[:], qT[:])
            nc.sync.dma_start(kt_sb[g][:, half:pos], kT[g, :, half:pos])

        def v_halves(g):
            nc.sync.dma_start(v_sb[g][:, :jh], vt[g, :, :jh])
            nc.sync.dma_start(v_sb[g][:, jh:n_full], vt[g, :, jh:n_full])

        for g in range(NG):
            kt_halves(g)
        for g in range(NG - 1):
            v_halves(g)
        nc.sync.dma_start(v_sb[gl][:, :jh], vt[gl, :, :jh])
        nc.sync.dma_start(v_sb[gl][:, jh:n_full - 4], vt[gl, :, jh:n_full - 4])
        nc.sync.dma_start(v_sb[gl][:, n_full - 4:n_full],
                          vt[gl, :, n_full - 4:n_full])

        # chunk list: (group, first_tile, n_full_tiles, is_tail);
        # tail first per group (opens the accumulation chains)
        chunks = []
        for g in range(NG):
            chunks.append((g, n_full, 0, True))
            for j0 in range(0, n_full, 8):
                chunks.append((g, j0, min(8, n_full - j0), False))

        den_ps = [None] * NG
        pv_ps = [None] * NG

        def emit_qk(c):
            g, j0, nt, is_tail = chunks[c]
            if is_tail:
                spt = ps_st.tile([NUM, QI], F32, tag="st", name=f"st{c}")
                nc.tensor.matmul(spt[:, :], kt_sb[g][:, pos:pos + NUM],
                                 q_sb[:, g * QI:(g + 1) * QI],
                                 start=True, stop=True)
                nc.vector.tensor_add(spt[:, :], spt[:, :], mb_sb[:, :])
                return spt
            spt = ps_s.tile([128, nt * QI], F32, tag="s", name=f"s{c}")
            for j in range(nt):
                nc.tensor.matmul(spt[:, j * QI:(j + 1) * QI],
                                 kt_sb[g][:, (j0 + j) * 128:(j0 + j + 1) * 128],
                                 q_sb[:, g * QI:(g + 1) * QI],
                                 start=True, stop=True)
            return spt

        def emit_exp(c, spt):
            g, j0, nt, is_tail = chunks[c]
            if is_tail:
                at = apool.tile([NUM, QI], F16, tag=f"at{c}", name=f"at{c}")
            else:
                at = apool.tile([128, nt * QI], F16, tag=f"a{c}", name=f"a{c}")
            nc.scalar.activation(at[:, :], spt[:, :], EXP, scale=scale)
            return at

        def emit_dv(c, at):
            g, j0, nt, is_tail = chunks[c]
            if den_ps[g] is None:
                den_ps[g] = ps_dn.tile([QI, 1], F32, tag="dn", name=f"dn{g}")
                pv_ps[g] = ps_pv.tile([HS, QI], F32, tag="pv", name=f"pv{g}")
            if is_tail:
                nc.tensor.matmul(den_ps[g][:, :], at[:, :], ones[:NUM, :],
                                 start=True, stop=False)
                nc.tensor.matmul(pv_ps[g][:, :], v_sb[g][:NUM, n_full, :],
                                 at[:, :], start=True, stop=False,
                                 skip_group_check=True)
                return
            for j in range(nt):
                sp = j0 + j == n_full - 1
                nc.tensor.matmul(den_ps[g][:, :],
                                 at[:, j * QI:(j + 1) * QI], ones[:, :],
                                 start=False, stop=sp,
                                 skip_group_check=True)
                nc.tensor.matmul(pv_ps[g][:, :], v_sb[g][:, j0 + j, :],
                                 at[:, j * QI:(j + 1) * QI],
                                 start=False, stop=sp,
                                 skip_group_check=True)

        def emit_norm(g):
            rec = npool.tile([QI, 1], F32, tag="rc")
            nc.vector.reciprocal(rec[:, :], den_ps[g][:, :])
            osb = npool.tile([HS, QI], F16, tag="os")
            nc.vector.tensor_copy(osb[:, :], pv_ps[g][:, :])
            ot = ps_ot.tile([QI, HS], F16, tag="ot", name=f"ot{g}")
            nc.tensor.transpose(ot[:, :], osb[:, :], id_sb[:, :])
            nc.vector.tensor_scalar_mul(out_sb[:, g, :], ot[:, :], rec[:, :])
            nc.sync.dma_start(out[g], out_sb[:, g, :])

        # two-pass emission: every engine is in-order, so pass 1 (QK+exp,
        # paced by the K^T stream) must not interleave with pass 2 (den/PV,
        # paced by the V stream) or late-V stalls would block later QKs.
        ats = []
        for c in range(len(chunks)):
            ats.append(emit_exp(c, emit_qk(c)))
        for c in range(len(chunks)):
            emit_dv(c, ats[c])
            g, j0, nt, is_tail = chunks[c]
            if not is_tail and j0 + nt == n_full:
                emit_norm(g)

    nc.compile()
    return nc


class _Runner:
    def __init__(self, nc):
        b2j.install_neuronx_cc_hook()
        self.nc = nc
        in_names, out_names, out_avals, zero_outs = [], [], [], []
        for alloc in nc.m.functions[0].allocations:
            if not isinstance(alloc, mybir.MemoryLocationSet):
                continue
            name = alloc.memorylocations[0].name
            if alloc.kind == "ExternalInput":
                in_names.append(name)
            elif alloc.kind == "ExternalOutput":
                out_names.append(name)
                shape = tuple(alloc.tensor_shape)
                dtype = mybir.dt.np(alloc.dtype)
                out_avals.append(jax.core.ShapedArray(shape, dtype))
                zero_outs.append(np.zeros(shape, dtype))
        part = nc.partition_id_tensor.name if nc.partition_id_tensor else None
        if part is not None:
            in_names = [n for n in in_names if n != part]
        self.in_names, self.out_names = in_names, out_names
        self.out_avals, self.zero_outs = out_avals, zero_outs
        all_names = in_names + out_names + ([part] if part else [])
        n_params = len(in_names)

        def _body(*args):
            operands = list(args)
            if part is not None:
                operands.append(b2j.partition_id_tensor())
            return tuple(b2j._bass_exec_p.bind(
                *operands, out_avals=tuple(out_avals), in_names=tuple(all_names),
                out_names=tuple(out_names), lowering_input_output_aliases=(),
                sim_require_finite=True, sim_require_nnan=True, nc=nc))

        devices = jax.devices()[:N_CORES]
        self.mesh = Mesh(np.asarray(devices), ("core",))
        in_specs = (PartitionSpec("core"),) * (n_params + len(out_names))
        out_specs = (PartitionSpec("core"),) * len(out_names)
        self.fn = jax.jit(shard_map(_body, mesh=self.mesh, in_specs=in_specs,
                                    out_specs=out_specs, check_rep=False),
                          keep_unused=True)

    def run(self, in_maps):
        sharding = jax.sharding.NamedSharding(self.mesh, PartitionSpec("core"))
        args = []
        for name in self.in_names:
            arr = np.concatenate([np.asarray(m[name]) for m in in_maps], axis=0)
            args.append(jax.device_put(arr, sharding))
        for z in self.zero_outs:
            args.append(jax.device_put(
                np.zeros((N_CORES * z.shape[0], *z.shape[1:]), z.dtype), sharding))
        outs = self.fn(*args)
        jax.block_until_ready(outs)
        return [{name: np.asarray(outs[i]).reshape(
            N_CORES, *self.out_avals[i].shape)[c]
            for i, name in enumerate(self.out_names)}
            for c in range(N_CORES)]


_cache = {}


def _get_runner(pos):
    if pos not in _cache:
        _cache[pos] = _Runner(build_program(pos))
    return _cache[pos]


def _make_maskb():
    # S^T tail tile [t_local, qi]: new token n=(qi%16) sees row p iff p<=n
    m = np.zeros((NUM, QI), np.float32)
    p = np.arange(NUM)[:, None]
    n = (np.arange(QI) % NUM)[None, :]
    m[p > n] = NEG
    return m


def _cast(x, cfg):
    dt, npdt = _DT[cfg[0]]
    s = cfg[1]
    if s != 1.0:
        x = np.asarray(x, np.float32) * s
    return np.asarray(x, np.float32).astype(npdt)


def kernel(query, key, value, k_cache, v_cache, input_pos):
    query = np.asarray(query, np.float32)
    key = np.asarray(key, np.float32)
    value = np.asarray(value, np.float32)
    k_cache = np.asarray(k_cache, np.float32)
    v_cache = np.asarray(v_cache, np.float32)
    pos = int(input_pos)
    T = pos + NUM
    n_vt = pos // 128 + 1

    runner = _get_runner(pos)
    ident = np.eye(128, dtype=np.float16)
    maskb = _make_maskb()

    in_maps = []
    for c in range(N_CORES):
        b = c // 2
        g0 = 4 * (c % 2)
        # q^T: [h, g, hd, tok] -> [HS, NG*QI]
        qs = query[b, g0 * 4:(g0 + NG) * 4]          # [16 heads, NUM, HS]
        qTh = np.ascontiguousarray(
            qs.reshape(NG * QI, HS).T).astype(np.float16)
        # K^T per group: [HS, T] with the new chunk appended
        kf = np.concatenate([k_cache[b, g0:g0 + NG, :pos], key[b, g0:g0 + NG]],
                            axis=1)                  # [NG, T, HS]
        kTh = _cast(np.ascontiguousarray(kf.transpose(0, 2, 1)), K_CFG)
        # V tiled: [NG, t%128, t//128, HS], zero-padded to n_vt*128 rows
        vf = np.concatenate([v_cache[b, g0:g0 + NG, :pos], value[b, g0:g0 + NG]],
                            axis=1)                  # [NG, T, HS]
        vp = np.zeros((NG, n_vt * 128, HS), np.float32)
        vp[:, :T] = vf
        vth = _cast(np.ascontiguousarray(
            vp.reshape(NG, n_vt, 128, HS).transpose(0, 2, 1, 3)), V_CFG)
        in_maps.append({"kT": kTh, "vt": vth, "qT": qTh,
                        "ident": ident, "maskb": maskb})

    results = runner.run(in_maps)

    full = np.empty((B, H, NUM, HS), np.float32)
    for c in range(N_CORES):
        b = c // 2
        g0 = 4 * (c % 2)
        full[b, g0 * 4:(g0 + NG) * 4] = results[c]["out"].reshape(16, NUM, HS)
    return np.ascontiguousarray(
        full.transpose(0, 2, 1, 3).reshape(B, NUM, H * HS))


# revision 25
# speedup vs baseline: 1.6132x; 1.0444x over previous
"""Trainium2 Bass kernel for DefaultKVCache attention (GQA decode-chunk).

Full-input contract: kernel(**inputs) takes the unsharded numpy inputs and
returns the full (B, NUM, H*HS) float32 output.

Problem shape (hardcoded):
  B=4, H=32, G=8 query groups (GQA 4 q-heads/group), HS=128,
  NUM=16 new tokens, cache length L=8192, input_pos (typically 4096).

Sharding: (batch, group-half) across 8 cores: core c -> b=c//2,
groups 4*(c%2)..4*(c%2)+4.  Fully local attention, no collectives.

Design (v2) — transposed-score orientation, host-side layout prep:
  - Host uploads K^T per group ([HS, T], new chunk concatenated) and V in
    SBUF-tiled layout ([t%128, t//128, HS], zero-padded), plus q^T.  All
    heavy operands are STATIONARY-side matmul inputs, so no on-device
    transposes of K or of the attention matrix are needed:
      S^T[t,qi]  = matmul(lhsT=K^T tile [h,t], rhs=q [h,qi])   (PSUM, f32)
      attn^T     = exp(scale*S^T)                              (Act -> f16)
      den[qi]    = matmul(lhsT=attn^T tile, rhs=ones [t,1])    (PSUM acc)
      out^T[h,qi]= matmul(lhsT=V tile [t,h], rhs=attn^T tile)  (PSUM acc)
    Final: transpose out^T via PE, multiply by 1/den per qi row, DMA out.
  - Only the last 16 cache rows need the causal mask (applied on the PSUM
    S^T tail tile with a DVE add before exp).
  - dtypes: q/attn/K fp16, V fp8-e3m4 with a 2x pre-scale folded into the
    softmax denominator (ones value = VSCALE).  Keeps rel-err ~1.2e-2 while
    halving V's DMA bytes.
"""
import sys
import numpy as np

for _p in ("/opt/trn_rl_repo", "/root/.axon_site/_ro/trn_rl_repo"):
    if _p not in sys.path:
        sys.path.insert(0, _p)

import ml_dtypes
from contextlib import ExitStack

import jax
from jax.sharding import Mesh, PartitionSpec
from jax.experimental.shard_map import shard_map

import concourse.bass as bass
from concourse import bacc, mybir, tile
import concourse.bass2jax as b2j

B, H, G, HS = 4, 32, 8, 128
NUM = 16
N_CORES = 8
NG = 4            # groups per core
QI = 64           # queries per group (4 heads x 16 tokens)
F32 = mybir.dt.float32
F16 = mybir.dt.float16
F8 = mybir.dt.float8e3       # e3m4
NEG = -1e30
EXP = mybir.ActivationFunctionType.Exp

# dtype knobs: "f16" or "f8" (fp8-e3m4, cast with the given pre-scale).
K_CFG = ("f8", 1.5)
V_CFG = ("f8", 2.0)

_DT = {"f16": (F16, np.float16), "f8": (F8, ml_dtypes.float8_e3m4)}


def build_program(pos):
    assert pos % 128 == 0 and NUM == 16
    T = pos + NUM
    n_full = pos // 128            # full 128-row K/V tiles
    n_vt = n_full + 1              # V tiles incl zero-padded tail tile
    scale = float(HS) ** -0.5 / float(K_CFG[1])   # K pre-scale folds in here
    kdt, _ = _DT[K_CFG[0]]
    vdt, _ = _DT[V_CFG[0]]

    nc = bacc.Bacc("TRN2", target_bir_lowering=False, debug=False,
                   enable_asserts=False, num_devices=N_CORES)
    kT = nc.dram_tensor("kT", [NG, HS, T], kdt, kind="ExternalInput").ap()
    vt = nc.dram_tensor("vt", [NG, 128, n_vt, HS], vdt,
                        kind="ExternalInput").ap()
    qT = nc.dram_tensor("qT", [HS, NG * QI], F16, kind="ExternalInput").ap()
    ident = nc.dram_tensor("ident", [128, 128], F32, kind="ExternalInput").ap()
    maskb = nc.dram_tensor("maskb", [NUM, QI], F32, kind="ExternalInput").ap()
    out = nc.dram_tensor("out", [NG, QI, HS], F32, kind="ExternalOutput").ap()

    with tile.TileContext(nc) as tc, ExitStack() as ctx:
        cpool = ctx.enter_context(tc.tile_pool(name="consts", bufs=1))
        apool = ctx.enter_context(tc.tile_pool(name="attn", bufs=4))
        npool = ctx.enter_context(tc.tile_pool(name="norm", bufs=2))
        ps_s = ctx.enter_context(tc.tile_pool(name="ps_s", bufs=2, space="PSUM"))
        ps_st = ctx.enter_context(tc.tile_pool(name="ps_st", bufs=1, space="PSUM"))
        ps_pv = ctx.enter_context(tc.tile_pool(name="ps_pv", bufs=1, space="PSUM"))
        ps_sm = ctx.enter_context(tc.tile_pool(name="ps_sm", bufs=2, space="PSUM"))

        # q first (needed by the first QK), then group 0's K/V, then the
        # remaining constants, then groups 1-3.  Every tile gets its own tag
        # so nothing shares a pool slot (a shared slot serializes the DMA
        # stream behind the previous group's compute).
        q_sb = cpool.tile([HS, NG * QI], F16, tag="q")
        ones = cpool.tile([128, 1], F16, tag="ones")
        nc.vector.memset(ones[:, :], float(V_CFG[1]))
        out_sb = cpool.tile([QI, NG, HS], F32, tag="out")

        # single SBUF tiles holding all 4 groups, so the tiny per-group tail
        # slices coalesce into ONE DMA each (every DMA instruction costs
        # ~625ns of serialized HWDGE descriptor-gen)
        kt_all = cpool.tile([HS, NG, T], kdt, tag="kt")
        v_all = cpool.tile([128, NG, n_vt, HS], vdt, tag="v")
        kt_sb = [kt_all[:, g] for g in range(NG)]
        v_sb = [v_all[:, g] for g in range(NG)]
        mb_sb = cpool.tile([NUM, QI], F32, tag="mb")
        id_sb = cpool.tile([128, 128], F32, tag="id")

        # small loads go on the Pool/SWDGE queue: their descriptor-gen runs
        # on the otherwise-idle Pool engine instead of the shared HWDGE.
        # Their transfers slot into DMA-engine gaps.
        nc.gpsimd.dma_start(kt_all[:, :, pos:],
                            kT[:, :, pos:].rearrange("g h t -> h g t"))
        nc.gpsimd.dma_start(mb_sb[:], maskb[:])
        nc.gpsimd.dma_start(v_all[:NUM, :, n_full, :],
                            vt[:, :NUM, n_full, :].rearrange("g p h -> p g h"))
        nc.gpsimd.dma_start(id_sb[:], ident[:])

        # bulk K/V halves per group on the SP/HWDGE queue; the stream ends
        # with a small V slice so the post-DMA dependency tail is short
        # Stream order: K^T always ~2 groups ahead of V so every group's
        # QK+exp completes during earlier transfers; the stream ends with
        # small V slices for the last group so the post-DMA tail is just
        # PV(last tiles) + normalize + out-DMA.
        half = (pos // 2) // 16 * 16
        jh = n_full // 2
        gl = NG - 1

        def kt_halves(g):
            nc.sync.dma_start(kt_sb[g][:, :half], kT[g, :, :half])
            if g == 0:
                nc.sync.dma_start(q_sb[:], qT[:])
            nc.sync.dma_start(kt_sb[g][:, half:pos], kT[g, :, half:pos])

        def v_halves(g):
            nc.sync.dma_start(v_sb[g][:, :jh], vt[g, :, :jh])
            nc.sync.dma_start(v_sb[g][:, jh:n_full], vt[g, :, jh:n_full])

        for g in range(NG):
            kt_halves(g)
        for g in range(NG - 1):
            v_halves(g)
        nc.sync.dma_start(v_sb[gl][:, :jh], vt[gl, :, :jh])
        nc.sync.dma_start(v_sb[gl][:, jh:n_full - 4], vt[gl, :, jh:n_full - 4])
        nc.sync.dma_start(v_sb[gl][:, n_full - 4:n_full],
                          vt[gl, :, n_full - 4:n_full])

        # chunk list: (group, first_tile, n_full_tiles, is_tail);
        # tail first per group (opens the accumulation chains).
        # 16-tile super-chunks: exp runs on [128, 1024] f32 PSUM (2 banks)
        # to amortize the Act engine's fixed per-instruction access latency.
        chunks = []
        for g in range(NG):
            chunks.append((g, n_full, 0, True))
            for j0 in range(0, n_full, 16):
                chunks.append((g, j0, min(16, n_full - j0), False))

        den_ps = [None] * NG
        pv_ps = [None] * NG
        ot_ps = [None] * NG
        # one PSUM bank holds every group's PV accumulator in alternating
        # halves: chains are strictly sequential (group g's chain stops and
        # is copied out before group g+2 reopens the same half), and a later
        # chain's start=True only clears has_written bits, not values.
        pv_all = ps_pv.tile([HS, 128], F32, tag="pv", name="pv_all")

        def emit_qk(c):
            g, j0, nt, is_tail = chunks[c]
            if is_tail:
                spt = ps_st.tile([NUM, QI], F32, tag="st", name=f"st{c}")
                spt = spt[:, :]
                nc.tensor.matmul(spt, kt_sb[g][:, pos:pos + NUM],
                                 q_sb[:, g * QI:(g + 1) * QI],
                                 start=True, stop=True)
                nc.vector.tensor_add(spt, spt, mb_sb[:, :])
                return spt
            spt = ps_s.tile([128, nt * QI], F32, tag="s", name=f"s{c}")
            for j in range(nt):
                nc.tensor.matmul(spt[:, j * QI:(j + 1) * QI],
                                 kt_sb[g][:, (j0 + j) * 128:(j0 + j + 1) * 128],
                                 q_sb[:, g * QI:(g + 1) * QI],
                                 start=True, stop=True)
            return spt

        def emit_exp(c, spt):
            g, j0, nt, is_tail = chunks[c]
            if is_tail:
                at = apool.tile([NUM, QI], F16, tag=f"at{c}", name=f"at{c}")
            else:
                at = apool.tile([128, nt * QI], F16, tag=f"a{c}", name=f"a{c}")
            nc.scalar.activation(at[:, :], spt, EXP, scale=scale)
            return at

        def emit_dv(c, at):
            g, j0, nt, is_tail = chunks[c]
            if den_ps[g] is None:
                # den accumulator and the transposed-out tile share one PSUM
                # bank: den's chain closes and is read (reciprocal) before
                # the transpose writes, and the next group's chain-opening
                # start=True happens after this group's normalize consumed
                # both (pool WAR deps enforce it).
                # den accumulator and the transposed-out tile share one
                # PSUM bank: den closes and is read (reciprocal) before the
                # transpose writes; the next group's chain-opening start=True
                # comes after this group's normalize consumed both.
                sm = ps_sm.tile([QI, 192], F32, tag="sm", name=f"sm{g}")
                den_ps[g] = sm[:, 0:1]
                ot_ps[g] = sm[:, 64:192]
                pv_ps[g] = pv_all[:, (g % 2) * QI:(g % 2) * QI + QI]
            if is_tail:
                nc.tensor.matmul(den_ps[g][:, :], at[:, :], ones[:NUM, :],
                                 start=True, stop=False)
                nc.tensor.matmul(pv_ps[g][:, :], v_sb[g][:NUM, n_full, :],
                                 at[:, :], start=True, stop=False,
                                 skip_group_check=True)
                return
            for j in range(nt):
                sp = j0 + j == n_full - 1
                nc.tensor.matmul(den_ps[g][:, :],
                                 at[:, j * QI:(j + 1) * QI], ones[:, :],
                                 start=False, stop=sp,
                                 skip_group_check=True)
                nc.tensor.matmul(pv_ps[g][:, :], v_sb[g][:, j0 + j, :],
                                 at[:, j * QI:(j + 1) * QI],
                                 start=False, stop=sp,
                                 skip_group_check=True)

        def emit_norm(g):
            rec = npool.tile([QI, 1], F32, tag="rc")
            nc.vector.reciprocal(rec[:, :], den_ps[g][:, :])
            osb = npool.tile([HS, QI], F32, tag="os")
            nc.vector.tensor_copy(osb[:, :], pv_ps[g][:, :])
            nc.tensor.transpose(ot_ps[g][:, :], osb[:, :], id_sb[:, :])
            nc.vector.tensor_scalar_mul(out_sb[:, g, :], ot_ps[g][:, :],
                                        rec[:, :])
            nc.sync.dma_start(out[g], out_sb[:, g, :])

        # Emission: engines execute in-order, so the QK+exp stream (paced by
        # K^T arrivals + PSUM-bank recycling) must stay ahead of the den/PV
        # stream (paced by V arrivals).  Emitting dv(g-2) right after group
        # g's QKs fills PE idle time without blocking any QK the banks
        # wouldn't have blocked anyway.
        ats = {}
        by_group = {g: [c for c, ch in enumerate(chunks) if ch[0] == g]
                    for g in range(NG)}

        def dv_group(g):
            for c in by_group[g]:
                emit_dv(c, ats[c])
            emit_norm(g)

        for g in range(NG):
            for c in by_group[g]:
                ats[c] = emit_exp(c, emit_qk(c))
        for g in range(NG):
            dv_group(g)

    nc.compile()
    return nc


class _Runner:
    def __init__(self, nc):
        b2j.install_neuronx_cc_hook()
        self.nc = nc
        in_names, out_names, out_avals, zero_outs = [], [], [], []
        for alloc in nc.m.functions[0].allocations:
            if not isinstance(alloc, mybir.MemoryLocationSet):
                continue
            name = alloc.memorylocations[0].name
            if alloc.kind == "ExternalInput":
                in_names.append(name)
            elif alloc.kind == "ExternalOutput":
                out_names.append(name)
                shape = tuple(alloc.tensor_shape)
                dtype = mybir.dt.np(alloc.dtype)
                out_avals.append(jax.core.ShapedArray(shape, dtype))
                zero_outs.append(np.zeros(shape, dtype))
        part = nc.partition_id_tensor.name if nc.partition_id_tensor else None
        if part is not None:
            in_names = [n for n in in_names if n != part]
        self.in_names, self.out_names = in_names, out_names
        self.out_avals, self.zero_outs = out_avals, zero_outs
        all_names = in_names + out_names + ([part] if part else [])
        n_params = len(in_names)

        def _body(*args):
            operands = list(args)
            if part is not None:
                operands.append(b2j.partition_id_tensor())
            return tuple(b2j._bass_exec_p.bind(
                *operands, out_avals=tuple(out_avals), in_names=tuple(all_names),
                out_names=tuple(out_names), lowering_input_output_aliases=(),
                sim_require_finite=True, sim_require_nnan=True, nc=nc))

        devices = jax.devices()[:N_CORES]
        self.mesh = Mesh(np.asarray(devices), ("core",))
        in_specs = (PartitionSpec("core"),) * (n_params + len(out_names))
        out_specs = (PartitionSpec("core"),) * len(out_names)
        self.fn = jax.jit(shard_map(_body, mesh=self.mesh, in_specs=in_specs,
                                    out_specs=out_specs, check_rep=False),
                          keep_unused=True)

    def run(self, in_maps):
        sharding = jax.sharding.NamedSharding(self.mesh, PartitionSpec("core"))
        args = []
        for name in self.in_names:
            arr = np.concatenate([np.asarray(m[name]) for m in in_maps], axis=0)
            args.append(jax.device_put(arr, sharding))
        for z in self.zero_outs:
            args.append(jax.device_put(
                np.zeros((N_CORES * z.shape[0], *z.shape[1:]), z.dtype), sharding))
        outs = self.fn(*args)
        jax.block_until_ready(outs)
        return [{name: np.asarray(outs[i]).reshape(
            N_CORES, *self.out_avals[i].shape)[c]
            for i, name in enumerate(self.out_names)}
            for c in range(N_CORES)]


_cache = {}


def _get_runner(pos):
    if pos not in _cache:
        _cache[pos] = _Runner(build_program(pos))
    return _cache[pos]


def _make_maskb():
    # S^T tail tile [t_local, qi]: new token n=(qi%16) sees row p iff p<=n
    m = np.zeros((NUM, QI), np.float32)
    p = np.arange(NUM)[:, None]
    n = (np.arange(QI) % NUM)[None, :]
    m[p > n] = NEG
    return m


def _cast(x, cfg):
    dt, npdt = _DT[cfg[0]]
    s = cfg[1]
    if s != 1.0:
        x = np.asarray(x, np.float32) * s
    return np.asarray(x, np.float32).astype(npdt)


def kernel(query, key, value, k_cache, v_cache, input_pos):
    query = np.asarray(query, np.float32)
    key = np.asarray(key, np.float32)
    value = np.asarray(value, np.float32)
    k_cache = np.asarray(k_cache, np.float32)
    v_cache = np.asarray(v_cache, np.float32)
    pos = int(input_pos)
    T = pos + NUM
    n_vt = pos // 128 + 1

    runner = _get_runner(pos)
    ident = np.eye(128, dtype=np.float32)
    maskb = _make_maskb()

    in_maps = []
    for c in range(N_CORES):
        b = c // 2
        g0 = 4 * (c % 2)
        # q^T: [h, g, hd, tok] -> [HS, NG*QI]
        qs = query[b, g0 * 4:(g0 + NG) * 4]          # [16 heads, NUM, HS]
        qTh = np.ascontiguousarray(
            qs.reshape(NG * QI, HS).T).astype(np.float16)
        # K^T per group: [HS, T] with the new chunk appended
        kf = np.concatenate([k_cache[b, g0:g0 + NG, :pos], key[b, g0:g0 + NG]],
                            axis=1)                  # [NG, T, HS]
        kTh = _cast(np.ascontiguousarray(kf.transpose(0, 2, 1)), K_CFG)
        # V tiled: [NG, t%128, t//128, HS], zero-padded to n_vt*128 rows
        vf = np.concatenate([v_cache[b, g0:g0 + NG, :pos], value[b, g0:g0 + NG]],
                            axis=1)                  # [NG, T, HS]
        vp = np.zeros((NG, n_vt * 128, HS), np.float32)
        vp[:, :T] = vf
        vth = _cast(np.ascontiguousarray(
            vp.reshape(NG, n_vt, 128, HS).transpose(0, 2, 1, 3)), V_CFG)
        in_maps.append({"kT": kTh, "vt": vth, "qT": qTh,
                        "ident": ident, "maskb": maskb})

    results = runner.run(in_maps)

    full = np.empty((B, H, NUM, HS), np.float32)
    for c in range(N_CORES):
        b = c // 2
        g0 = 4 * (c % 2)
        full[b, g0 * 4:(g0 + NG) * 4] = results[c]["out"].reshape(16, NUM, HS)
    return np.ascontiguousarray(
        full.transpose(0, 2, 1, 3).reshape(B, NUM, H * HS))
